# revision 24
# baseline (speedup 1.0000x reference)
"""CoWindowAttention Trainium2 kernel — 8-core data-parallel Bass/Tile.

Feature-major layout ([feature, token] in SBUF) so PE contractions never
need on-chip transposes.  Host pre-transposes big I/O, folds W1@Wq (only q
of big is used), folds the softmax scale into the weights, moves the
epilogue bias (bv@W2+b2) to the host-side gather, drops the k bias
(softmax-shift invariant), and ships exp(rel-pos bias) so the kernel
multiplies instead of add-then-exp.

Scores are computed transposed and HEAD-PAIR PACKED: for pair t (heads
2t,2t+1) and window g, the stationary kstat[:, (t,g)-block] holds head
2t's keys in cols 0-63 (rows = feature band 32*2t) and head 2t+1's keys
in cols 64-127 (band 32*(2t+1)); one N=256 matmul then yields both heads'
scores for all 256 queries of window g: sp[(h2,k), (t,g,q)].  The u
matmuls are packed the same way via vstat (v with keys on partitions,
duplicated into both 64-row bands so each (t,g) block contracts over
(h2,k)), accumulating the two pairs into full 128-feature outputs.
Softmax: z = ones-matmul over es, 1/z via reciprocal_approx_fast (DVE),
broadcast to feature partitions with a tiny K=4 f32r matmul.

All matmul operands are bf16 (f32r for the 1/z broadcast); PSUM f32.
Output is written bf16 (halves write traffic; tolerance is 2e-2).
Engine placement: PE matmuls; Act = qb drain + exp + out drain;
DVE = k/v scatters + recip + u-normalize; GpSimd = bias multiply
(SBUF-only — gpsimd has no PSUM port).  The group loop is software-
pipelined 3 deep with stage emissions interleaved (A(i+2) | C2(i-1) |
B(i+1) | C1(i)) so each engine always has ready work.
"""

import sys
import numpy as np

if "/opt/trn_rl_repo" not in sys.path:
    sys.path.insert(0, "/opt/trn_rl_repo")

from contextlib import ExitStack

from concourse import bacc, bass, tile, mybir
from concourse.bass_utils import run_bass_kernel_spmd

W_, WU, H, SF, BF, HD = 8, 16, 4, 128, 256, 32
NB, NS = WU * WU, W_ * W_          # 256, 64
B, NCORES = 1024, 8
BLOC = B // NCORES
G = 2
NGRP = BLOC // G
import os as _os
NGRP_RUN = int(_os.environ.get("KNGRP", NGRP))
SCALE = HD ** -0.5

F32 = mybir.dt.float32
F32R = mybir.dt.float32r
BF16 = mybir.dt.bfloat16
AF = mybir.ActivationFunctionType
ALU = mybir.AluOpType


def _rel_pos_index():
    ch, cw = np.meshgrid(np.arange(WU), np.arange(WU), indexing="ij")
    big = np.stack([ch.reshape(-1), cw.reshape(-1)])
    sh, sw = np.meshgrid(np.arange(W_), np.arange(W_), indexing="ij")
    small = np.stack([sh.reshape(-1), sw.reshape(-1)])
    rel = big[:, :, None] - small[:, None, :]
    return (rel[0] + W_ - 1) * (2 * W_ - 1) + (rel[1] + W_ - 1)   # (NB, NS)


def build_nc():
    nc = bacc.Bacc("TRN2", target_bir_lowering=False, debug=False,
                   enable_asserts=False)

    bigxT = nc.dram_tensor("bigxT", (BLOC, BF, NB), BF16, kind="ExternalInput").ap()
    smallxT = nc.dram_tensor("smallxT", (BLOC, SF, NS), BF16, kind="ExternalInput").ap()
    wbq_d = nc.dram_tensor("wbq", (BF, SF), BF16, kind="ExternalInput").ap()
    wk_d = nc.dram_tensor("wk", (SF, SF), BF16, kind="ExternalInput").ap()
    wv_d = nc.dram_tensor("wv", (SF, SF), BF16, kind="ExternalInput").ap()
    w2_d = nc.dram_tensor("w2", (SF, BF), BF16, kind="ExternalInput").ap()
    expb_d = nc.dram_tensor("expb", (128, 1024), BF16, kind="ExternalInput").ap()
    ones4_d = nc.dram_tensor("ones4", (128, 256), BF16, kind="ExternalInput").ap()
    bvec_d = nc.dram_tensor("bvec", (128, 1), F32, kind="ExternalInput").ap()
    zeros_d = nc.dram_tensor("zeros", (128, 512), BF16, kind="ExternalInput").ap()
    outT = nc.dram_tensor("outT", (BLOC, BF, NB), BF16, kind="ExternalOutput").ap()

    with ExitStack() as ctx:
        ctx.enter_context(nc.allow_low_precision(reason="bf16 matmul inputs"))
        tc = ctx.enter_context(tile.TileContext(nc))
        wp = ctx.enter_context(tc.tile_pool(name="w", bufs=1))
        sb = ctx.enter_context(tc.tile_pool(name="sb", bufs=3))
        ps = ctx.enter_context(tc.tile_pool(name="ps", bufs=1, space="PSUM"))

        wbq = wp.tile([128, 256], BF16)
        nc.sync.dma_start(wbq[:].rearrange("p (c m) -> p c m", c=2),
                          wbq_d.rearrange("(c p) m -> p c m", p=128))
        wk = wp.tile([128, 128], BF16)
        nc.sync.dma_start(wk[:], wk_d)
        wv = wp.tile([128, 128], BF16)
        nc.sync.dma_start(wv[:], wv_d)
        w2 = wp.tile([128, 256], BF16)
        nc.sync.dma_start(w2[:], w2_d)
        expb = wp.tile([128, 1024], BF16)
        nc.sync.dma_start(expb[:], expb_d)
        ones4 = wp.tile([128, 256], BF16)
        nc.sync.dma_start(ones4[:], ones4_d)
        bvec = wp.tile([128, 1], F32)
        nc.sync.dma_start(bvec[:], bvec_d)
        # persistent zero-padded stationaries (manually cycled)
        kstats, vstats = [], []
        for i in range(2):
            kt = wp.tile([128, 512], BF16, name=f"kstat{i}")
            nc.sync.dma_start(kt[:, 0:512], zeros_d)
            kstats.append(kt)
        for i in range(3):
            vt = wp.tile([128, 512], BF16, name=f"vstat{i}")
            nc.sync.dma_start(vt[:, 0:512], zeros_d)
            vstats.append(vt)

        st = {}   # per-group in-flight state

        def stage_load(i):
            """DMA loads for group i (issued well ahead so PE never waits)"""
            b0 = i * G
            # big cols: g*512 + c*256 + n (g outermost so the DMA's (g,c)
            # blocks land on an arithmetic DRAM stride and balance to 3 dims)
            big = sb.tile([128, 1024], BF16, tag="big", name=f"big{i}")
            nc.sync.dma_start(
                big[:].rearrange("p (g c n) -> p g c n", g=G, c=2),
                bigxT[b0:b0 + G].rearrange("g (c p) n -> p g c n", p=128))
            small = sb.tile([128, 128], BF16, tag="small", name=f"small{i}")
            nc.sync.dma_start(
                small[:].rearrange("p (g n) -> p g n", g=G),
                smallxT[b0:b0 + G].rearrange("g p n -> p g n"))
            st[i] = dict(big=big, small=small)

        def stage_proj(i):
            """project q/k/v for group i"""
            g_ = st[i]
            big, small = g_.pop("big"), g_.pop("small")
            kstat, vstat = kstats[i % 2], vstats[i % 3]
            qb_ps = ps.tile([128, 512], F32, tag="qb_ps", name=f"qbp{i}")
            for c in range(2):
                nc.tensor.matmul(qb_ps[:], wbq[:, c * 128:(c + 1) * 128],
                                 bass.AP(big.tensor, big.offset + 256 * c,
                                         [[1024, 128], [512, G], [1, 256]]),
                                 start=(c == 0), stop=(c == 1))
            # kv_ps cols: 0:128 = k [feat, (g,key)]; 128:256 = v1 [(g,key), feat];
            # 256:384 = v2 [(1-g,key), feat]
            kv_ps = ps.tile([128, 512], F32, tag="kv_ps", name=f"kvp{i}")
            nc.tensor.matmul(kv_ps[:, 0:128], wk[:], small[:], start=True, stop=True)
            nc.tensor.matmul(kv_ps[:, 128:256], small[:], wv[:], start=True, stop=True)
            nc.tensor.matmul(kv_ps[0:64, 256:384], small[:, 64:128], wv[:],
                             start=True, stop=True)
            nc.tensor.matmul(kv_ps[64:128, 256:384], small[:, 0:64], wv[:],
                             start=True, stop=True)

            qb = sb.tile([128, 512], BF16, tag="qb", name=f"qb{i}")
            nc.scalar.activation(qb[:], qb_ps[:], AF.Identity, bias=bvec[:, 0:1])

            # k scatter: band h rows of kv_ps k -> kstat col (h//2)*256+g*128+(h%2)*64
            for h in range(4):
                nc.vector.tensor_copy(
                    bass.AP(kstat.tensor,
                            kstat.offset + 32 * h * 512 + (h // 2) * 256 + (h % 2) * 64,
                            [[512, 32], [128, 2], [1, 64]]),
                    kv_ps[32 * h:32 * h + 32, 0:128].rearrange("p (g k) -> p g k", g=2))
            # v scatter: vstat block (t,g) col 256t+128g holds, at rows 64*h2+key,
            # cols 64t+32*h2+d, the v features of head 2t+h2 for window g.
            # (src_col, dst_col, part_lo) per copy; t handled inside the AP.
            for src0, dst0, p0 in ((128, 0, 0), (160, 160, 64),
                                   (256, 128, 0), (288, 32, 64)):
                nc.vector.tensor_copy(
                    bass.AP(vstat.tensor, vstat.offset + p0 * 512 + dst0,
                            [[512, 64], [320, 2], [1, 32]]),
                    bass.AP(kv_ps.tensor, kv_ps.offset + p0 * 512 + src0,
                            [[512, 64], [64, 2], [1, 32]]))
            g_.update(qb=qb, kstat=kstat, vstat=vstat)

        def stage_b(i):
            """scores + exp + bias-mult for group i"""
            g_ = st[i]
            qb, kstat = g_["qb"], g_["kstat"]
            sp = ps.tile([128, 1024], F32, tag="sp", name=f"sp{i}")
            for t in range(2):
                for g in range(G):
                    nc.tensor.matmul(
                        sp[:, t * 512 + g * 256:t * 512 + g * 256 + 256],
                        kstat[:, t * 256 + g * 128:t * 256 + (g + 1) * 128],
                        qb[:, g * 256:(g + 1) * 256],
                        start=True, stop=True)
            es0 = sb.tile([128, 1024], BF16, tag="es0", bufs=2, name=f"es0_{i}")
            nc.scalar.activation(es0[:], sp[:], AF.Exp)
            es = sb.tile([128, 1024], BF16, tag="es", name=f"es{i}")
            nc.gpsimd.tensor_tensor(es[:], es0[:], expb[:], ALU.mult)
            g_["es"] = es

        def stage_c1(i):
            """softmax denominators + normalize for group i"""
            g_ = st[i]
            es, vstat = g_["es"], g_["vstat"]
            # z broadcast to all feature partitions directly: ones4 col
            # (t*128+f) has 1s at rows 64*h2..64*h2+64 for h2 = f//32 - 2t
            # (zero outside pair t's feature quarter), so accumulating the
            # two pair matmuls yields zfull[f, (g,q)] = z[head f//32, g, q].
            z_ps = ps.tile([128, 512], F32, tag="z", name=f"z{i}")
            for t in range(2):
                nc.tensor.matmul(z_ps[:], ones4[:, t * 128:(t + 1) * 128],
                                 es[:, t * 512:(t + 1) * 512],
                                 start=(t == 0), stop=(t == 1))
            rz = sb.tile([128, 512], F32, tag="rz", bufs=2, name=f"rz{i}")
            nc.vector.reciprocal_approx_fast(out=rz[:], in_=z_ps[:])
            u_ps = ps.tile([128, 512], F32, tag="u", name=f"u{i}")
            for g in range(G):
                for t in range(2):
                    nc.tensor.matmul(
                        u_ps[:, g * 256:(g + 1) * 256],
                        vstat[:, t * 256 + g * 128:t * 256 + (g + 1) * 128],
                        es[:, t * 512 + g * 256:t * 512 + g * 256 + 256],
                        start=(t == 0), stop=(t == 1))
            g_.update(rz=rz, u_ps=u_ps)

        def stage_c2(i):
            """normalize u for group i"""
            g_ = st[i]
            rz, u_ps = g_.pop("rz"), g_.pop("u_ps")
            un = sb.tile([128, 512], BF16, tag="un", bufs=2, name=f"un{i}")
            nc.vector.tensor_tensor(un[:], u_ps[:], rz[:], ALU.mult)
            g_["un"] = un

        def stage_c3(i):
            """final projection + output for group i"""
            g_ = st.pop(i)
            un = g_["un"]
            b0 = i * G
            p_ps = ps.tile([128, 1024], F32, tag="p", name=f"p{i}")
            for c in range(2):
                nc.tensor.matmul(p_ps[:, c * 512:(c + 1) * 512],
                                 w2[:, c * 128:(c + 1) * 128], un[:],
                                 start=True, stop=True)
            # out_sb cols (g, c, n); the Act drain does the (c,g)->(g,c)
            # reorder so the DMA balances to 3 dims
            out_sb = sb.tile([128, 1024], BF16, tag="out", bufs=2, name=f"out{i}")
            nc.scalar.activation(
                out_sb[:].rearrange("p (g c n) -> p g c n", g=G, c=2),
                p_ps[:].rearrange("p (c g n) -> p g c n", c=2, g=G),
                AF.Identity)
            nc.sync.dma_start(
                outT[b0:b0 + G].rearrange("g (c p) n -> p g c n", p=128),
                out_sb[:].rearrange("p (g c n) -> p g c n", g=G, c=2))

        # software pipeline; emission order per iter =
        #   load(i+3) | proj(i+2) | c2(i-1) | c3(i-2) | b(i+1) | c1(i)
        # so every PE instruction's inputs were produced >= 1 iteration
        # earlier and the in-order PE queue never waits on DMA or slow
        # cross-engine chains.
        N = NGRP_RUN
        for j in range(min(3, N)):
            stage_load(j)
        stage_proj(0)
        if N > 1:
            stage_proj(1)
        stage_b(0)
        for i in range(N):
            if i + 3 < N:
                stage_load(i + 3)
            if i + 2 < N:
                stage_proj(i + 2)
            if i >= 1:
                stage_c2(i - 1)
            if i >= 2:
                stage_c3(i - 2)
            if i + 1 < N:
                stage_b(i + 1)
            stage_c1(i)
        stage_c2(N - 1)
        if N >= 2:
            stage_c3(N - 2)
        stage_c3(N - 1)

    nc.compile()
    return nc


_NC = None


def _get_nc():
    global _NC
    if _NC is None:
        _NC = build_nc()
    return _NC


def _host_consts(W1, b1, Wqkv, bqkv, W2, b2, bias_table):
    import ml_dtypes
    BFnp = ml_dtypes.bfloat16
    Wq, Wk, Wv = Wqkv[:, :SF], Wqkv[:, SF:2 * SF], Wqkv[:, 2 * SF:]
    bq, bv = bqkv[:SF], bqkv[2 * SF:]
    # k bias (bqkv[SF:2SF]) shifts every score of a query by the same amount
    # per (q, h) -> softmax invariant -> dropped.
    wbq = (W1 @ Wq) * SCALE
    bbq = (b1 @ Wq + bq) * SCALE
    c2 = (bv @ W2 + b2).astype(np.float32)
    bias = bias_table[_rel_pos_index()]            # (NB=q, NS=k, H)
    # expb[64*h2+k, t*512+g*256+q] = exp(bias[q, k, 2t+h2]), same for both g
    biasT = np.zeros((128, 1024), np.float32)
    for t in range(2):
        for h2 in range(2):
            bT = bias[:, :, 2 * t + h2].T          # (k, q)
            for g in range(G):
                biasT[64 * h2:64 * h2 + 64,
                      t * 512 + g * 256:t * 512 + (g + 1) * 256] = bT
    # ones4[64*h2:64*(h2+1), t*128 + 32*h : t*128 + 32*h + 32] = 1 for
    # h = 2t + h2: the z matmul for pair t then writes z[head f//32] to
    # every feature partition f of that pair's quarter.
    ones4 = np.zeros((128, 256), np.float32)
    for t in range(2):
        for h2 in range(2):
            h = 2 * t + h2
            ones4[64 * h2:64 * h2 + 64, t * 128 + 32 * h:t * 128 + 32 * h + 32] = 1.0
    bvec = np.zeros((128, 1), np.float32)
    bvec[:, 0] = bbq
    consts = dict(wbq=np.ascontiguousarray(wbq.astype(BFnp)),
                  wk=np.ascontiguousarray(Wk.astype(BFnp)),
                  wv=np.ascontiguousarray(Wv.astype(BFnp)),
                  w2=np.ascontiguousarray(W2.astype(BFnp)),
                  expb=np.exp(biasT).astype(BFnp),
                  ones4=ones4.astype(BFnp),
                  bvec=bvec, zeros=np.zeros((128, 512), BFnp))
    return consts, c2


def make_in_maps(big_x, small_x, W1, b1, Wqkv, bqkv, W2, b2, bias_table):
    import ml_dtypes
    BFnp = ml_dtypes.bfloat16
    consts, c2 = _host_consts(
        np.asarray(W1, np.float32), np.asarray(b1, np.float32),
        np.asarray(Wqkv, np.float32), np.asarray(bqkv, np.float32),
        np.asarray(W2, np.float32), np.asarray(b2, np.float32),
        np.asarray(bias_table, np.float32))
    big_x = np.asarray(big_x, np.float32)
    small_x = np.asarray(small_x, np.float32)
    in_maps = []
    for c in range(NCORES):
        sl = slice(c * BLOC, (c + 1) * BLOC)
        m = dict(consts)
        m["bigxT"] = np.ascontiguousarray(big_x[sl].transpose(0, 2, 1).astype(BFnp))
        m["smallxT"] = np.ascontiguousarray(small_x[sl].transpose(0, 2, 1).astype(BFnp))
        in_maps.append(m)
    return in_maps, c2


def gather_out(results, c2):
    # outT is (BLOC, BF, NB) bf16; epilogue bias c2 (per BF feature) added here
    outs = [(r["outT"].astype(np.float32) + c2[None, :, None]).transpose(0, 2, 1)
            for r in results]
    return np.ascontiguousarray(np.concatenate(outs, axis=0), dtype=np.float32)


def run(inputs, **kw):
    nc = _get_nc()
    in_maps, c2 = make_in_maps(**inputs)
    res = run_bass_kernel_spmd(nc, in_maps, core_ids=list(range(NCORES)), **kw)
    res.c2 = c2
    return res


def kernel(**inputs):
    res = run(inputs)
    return gather_out(res.results, res.c2)


# revision 25
# speedup vs baseline: 1.2983x; 1.2983x over previous
"""CoWindowAttention Trainium2 kernel — 8-core data-parallel Bass/Tile.

Feature-major layout ([feature, token] in SBUF) so PE contractions never
need on-chip transposes.  Host pre-transposes big I/O, folds W1@Wq (only q
of big is used), folds the softmax scale into the weights, moves the
epilogue bias (bv@W2+b2) to the host-side gather, drops the k bias
(softmax-shift invariant), and ships exp(rel-pos bias) so the kernel
multiplies instead of add-then-exp.

Scores are computed transposed and HEAD-PAIR PACKED: for pair t (heads
2t,2t+1) and window g, the stationary kstat[:, (t,g)-block] holds head
2t's keys in cols 0-63 (rows = feature band 32*2t) and head 2t+1's keys
in cols 64-127 (band 32*(2t+1)); one N=256 matmul then yields both heads'
scores for all 256 queries of window g: sp[(h2,k), (t,g,q)].  The u
matmuls are packed the same way via vstat (v with keys on partitions,
duplicated into both 64-row bands so each (t,g) block contracts over
(h2,k)), accumulating the two pairs into full 128-feature outputs.
Softmax: z = ones-matmul over es, 1/z via reciprocal_approx_fast (DVE),
broadcast to feature partitions with a tiny K=4 f32r matmul.

All matmul operands are bf16 (f32r for the 1/z broadcast); PSUM f32.
Output is written bf16 (halves write traffic; tolerance is 2e-2).
Engine placement: PE matmuls; Act = qb drain + exp + out drain;
DVE = k/v scatters + recip + u-normalize; GpSimd = bias multiply
(SBUF-only — gpsimd has no PSUM port).  The group loop is software-
pipelined 3 deep with stage emissions interleaved (A(i+2) | C2(i-1) |
B(i+1) | C1(i)) so each engine always has ready work.
"""

import sys
import numpy as np

if "/opt/trn_rl_repo" not in sys.path:
    sys.path.insert(0, "/opt/trn_rl_repo")

from contextlib import ExitStack

from concourse import bacc, bass, tile, mybir
from concourse.bass_utils import run_bass_kernel_spmd

W_, WU, H, SF, BF, HD = 8, 16, 4, 128, 256, 32
NB, NS = WU * WU, W_ * W_          # 256, 64
B, NCORES = 1024, 8
BLOC = B // NCORES
G = 2
NGRP = BLOC // G
import os as _os
NGRP_RUN = int(_os.environ.get("KNGRP", NGRP))
SCALE = HD ** -0.5

F32 = mybir.dt.float32
F32R = mybir.dt.float32r
BF16 = mybir.dt.bfloat16
AF = mybir.ActivationFunctionType
ALU = mybir.AluOpType


def _rel_pos_index():
    ch, cw = np.meshgrid(np.arange(WU), np.arange(WU), indexing="ij")
    big = np.stack([ch.reshape(-1), cw.reshape(-1)])
    sh, sw = np.meshgrid(np.arange(W_), np.arange(W_), indexing="ij")
    small = np.stack([sh.reshape(-1), sw.reshape(-1)])
    rel = big[:, :, None] - small[:, None, :]
    return (rel[0] + W_ - 1) * (2 * W_ - 1) + (rel[1] + W_ - 1)   # (NB, NS)


def build_nc():
    nc = bacc.Bacc("TRN2", target_bir_lowering=False, debug=False,
                   enable_asserts=False)

    bigxT = nc.dram_tensor("bigxT", (BLOC, BF, NB), BF16, kind="ExternalInput").ap()
    smallxT = nc.dram_tensor("smallxT", (BLOC, SF, NS), BF16, kind="ExternalInput").ap()
    wbq_d = nc.dram_tensor("wbq", (BF, SF), BF16, kind="ExternalInput").ap()
    wk_d = nc.dram_tensor("wk", (SF, SF), BF16, kind="ExternalInput").ap()
    wv_d = nc.dram_tensor("wv", (SF, SF), BF16, kind="ExternalInput").ap()
    w2_d = nc.dram_tensor("w2", (SF, BF), BF16, kind="ExternalInput").ap()
    expb_d = nc.dram_tensor("expb", (128, 1024), BF16, kind="ExternalInput").ap()
    ones4_d = nc.dram_tensor("ones4", (128, 256), BF16, kind="ExternalInput").ap()
    bvec_d = nc.dram_tensor("bvec", (128, 1), F32, kind="ExternalInput").ap()
    zeros_d = nc.dram_tensor("zeros", (128, 512), BF16, kind="ExternalInput").ap()
    outT = nc.dram_tensor("outT", (BLOC, BF, NB), BF16, kind="ExternalOutput").ap()

    with ExitStack() as ctx:
        ctx.enter_context(nc.allow_low_precision(reason="bf16 matmul inputs"))
        tc = ctx.enter_context(tile.TileContext(nc))
        wp = ctx.enter_context(tc.tile_pool(name="w", bufs=1))
        sb = ctx.enter_context(tc.tile_pool(name="sb", bufs=3))
        ps = ctx.enter_context(tc.tile_pool(name="ps", bufs=1, space="PSUM"))

        wbq = wp.tile([128, 256], BF16)
        nc.sync.dma_start(wbq[:].rearrange("p (c m) -> p c m", c=2),
                          wbq_d.rearrange("(c p) m -> p c m", p=128))
        wk = wp.tile([128, 128], BF16)
        nc.sync.dma_start(wk[:], wk_d)
        wv = wp.tile([128, 128], BF16)
        nc.sync.dma_start(wv[:], wv_d)
        w2 = wp.tile([128, 256], BF16)
        nc.sync.dma_start(w2[:], w2_d)
        expb = wp.tile([128, 1024], BF16)
        nc.sync.dma_start(expb[:], expb_d)
        ones4 = wp.tile([128, 256], BF16)
        nc.sync.dma_start(ones4[:], ones4_d)
        bvec = wp.tile([128, 1], F32)
        nc.sync.dma_start(bvec[:], bvec_d)
        # persistent zero-padded stationaries (manually cycled)
        kstats, vstats = [], []
        for i in range(2):
            kt = wp.tile([128, 512], BF16, name=f"kstat{i}")
            nc.sync.dma_start(kt[:, 0:512], zeros_d)
            kstats.append(kt)
        for i in range(3):
            vt = wp.tile([128, 512], BF16, name=f"vstat{i}")
            nc.sync.dma_start(vt[:, 0:512], zeros_d)
            vstats.append(vt)

        st = {}   # per-group in-flight state

        def stage_load(i):
            """DMA loads for group i (issued well ahead so PE never waits)"""
            b0 = i * G
            # big cols: g*512 + c*256 + n (g outermost so the DMA's (g,c)
            # blocks land on an arithmetic DRAM stride and balance to 3 dims)
            big = sb.tile([128, 1024], BF16, tag="big", name=f"big{i}")
            nc.sync.dma_start(
                big[:].rearrange("p (g c n) -> p g c n", g=G, c=2),
                bigxT[b0:b0 + G].rearrange("g (c p) n -> p g c n", p=128))
            small = sb.tile([128, 128], BF16, tag="small", name=f"small{i}")
            nc.sync.dma_start(
                small[:].rearrange("p (g n) -> p g n", g=G),
                smallxT[b0:b0 + G].rearrange("g p n -> p g n"))
            st[i] = dict(big=big, small=small)

        def stage_proj(i):
            """project q/k/v for group i"""
            g_ = st[i]
            big, small = g_.pop("big"), g_.pop("small")
            kstat, vstat = kstats[i % 2], vstats[i % 3]
            qb_ps = ps.tile([128, 512], F32, tag="qb_ps", name=f"qbp{i}")
            for c in range(2):
                nc.tensor.matmul(qb_ps[:], wbq[:, c * 128:(c + 1) * 128],
                                 bass.AP(big.tensor, big.offset + 256 * c,
                                         [[1024, 128], [512, G], [1, 256]]),
                                 start=(c == 0), stop=(c == 1))
            # kv_ps cols: 0:128 = k [feat, (g,key)]; 128:256 = v1 [(g,key), feat];
            # 256:384 = v2 [(1-g,key), feat]
            kv_ps = ps.tile([128, 512], F32, tag="kv_ps", name=f"kvp{i}")
            nc.tensor.matmul(kv_ps[:, 0:128], wk[:], small[:], start=True, stop=True)
            nc.tensor.matmul(kv_ps[:, 128:256], small[:], wv[:], start=True, stop=True)
            nc.tensor.matmul(kv_ps[0:64, 256:384], small[:, 64:128], wv[:],
                             start=True, stop=True)
            nc.tensor.matmul(kv_ps[64:128, 256:384], small[:, 0:64], wv[:],
                             start=True, stop=True)

            qb = sb.tile([128, 512], BF16, tag="qb", name=f"qb{i}")
            nc.scalar.activation(qb[:], qb_ps[:], AF.Identity, bias=bvec[:, 0:1])

            # k scatter: band h rows of kv_ps k -> kstat col (h//2)*256+g*128+(h%2)*64
            for h in range(4):
                nc.vector.tensor_copy(
                    bass.AP(kstat.tensor,
                            kstat.offset + 32 * h * 512 + (h // 2) * 256 + (h % 2) * 64,
                            [[512, 32], [128, 2], [1, 64]]),
                    kv_ps[32 * h:32 * h + 32, 0:128].rearrange("p (g k) -> p g k", g=2))
            # v scatter: vstat block (t,g) col 256t+128g holds, at rows 64*h2+key,
            # cols 64t+32*h2+d, the v features of head 2t+h2 for window g.
            # (src_col, dst_col, part_lo) per copy; t handled inside the AP.
            for src0, dst0, p0 in ((128, 0, 0), (160, 160, 64),
                                   (256, 128, 0), (288, 32, 64)):
                nc.vector.tensor_copy(
                    bass.AP(vstat.tensor, vstat.offset + p0 * 512 + dst0,
                            [[512, 64], [320, 2], [1, 32]]),
                    bass.AP(kv_ps.tensor, kv_ps.offset + p0 * 512 + src0,
                            [[512, 64], [64, 2], [1, 32]]))
            g_.update(qb=qb, kstat=kstat, vstat=vstat)

        def stage_b(i):
            """scores + exp + bias-mult for group i"""
            g_ = st[i]
            qb, kstat = g_["qb"], g_["kstat"]
            sp = ps.tile([128, 1024], F32, tag="sp", name=f"sp{i}")
            for t in range(2):
                for g in range(G):
                    nc.tensor.matmul(
                        sp[:, t * 512 + g * 256:t * 512 + g * 256 + 256],
                        kstat[:, t * 256 + g * 128:t * 256 + (g + 1) * 128],
                        qb[:, g * 256:(g + 1) * 256],
                        start=True, stop=True)
            es0 = sb.tile([128, 1024], BF16, tag="es0", bufs=2, name=f"es0_{i}")
            nc.scalar.activation(es0[:], sp[:], AF.Exp)
            # DVE, not gpsimd: gpsimd semaphore ops cost ~3us each and poison
            # the es -> z/u critical path
            es = sb.tile([128, 1024], BF16, tag="es", name=f"es{i}")
            nc.vector.tensor_tensor(es[:], es0[:], expb[:], ALU.mult)
            g_["es"] = es

        def stage_c1(i):
            """softmax denominators + normalize for group i"""
            g_ = st[i]
            es, vstat = g_["es"], g_["vstat"]
            # z broadcast to all feature partitions directly: ones4 col
            # (t*128+f) has 1s at rows 64*h2..64*h2+64 for h2 = f//32 - 2t
            # (zero outside pair t's feature quarter), so accumulating the
            # two pair matmuls yields zfull[f, (g,q)] = z[head f//32, g, q].
            z_ps = ps.tile([128, 512], F32, tag="z", name=f"z{i}")
            for t in range(2):
                nc.tensor.matmul(z_ps[:], ones4[:, t * 128:(t + 1) * 128],
                                 es[:, t * 512:(t + 1) * 512],
                                 start=(t == 0), stop=(t == 1))
            rz = sb.tile([128, 512], F32, tag="rz", bufs=2, name=f"rz{i}")
            nc.vector.reciprocal_approx_fast(out=rz[:], in_=z_ps[:])
            u_ps = ps.tile([128, 512], F32, tag="u", name=f"u{i}")
            for g in range(G):
                for t in range(2):
                    nc.tensor.matmul(
                        u_ps[:, g * 256:(g + 1) * 256],
                        vstat[:, t * 256 + g * 128:t * 256 + (g + 1) * 128],
                        es[:, t * 512 + g * 256:t * 512 + g * 256 + 256],
                        start=(t == 0), stop=(t == 1))
            g_.update(rz=rz, u_ps=u_ps)

        def stage_c2(i):
            """normalize u for group i"""
            g_ = st[i]
            rz, u_ps = g_.pop("rz"), g_.pop("u_ps")
            un = sb.tile([128, 512], BF16, tag="un", bufs=2, name=f"un{i}")
            nc.vector.tensor_tensor(un[:], u_ps[:], rz[:], ALU.mult)
            g_["un"] = un

        def stage_c3(i):
            """final projection + output for group i"""
            g_ = st.pop(i)
            un = g_["un"]
            b0 = i * G
            p_ps = ps.tile([128, 1024], F32, tag="p", name=f"p{i}")
            for c in range(2):
                nc.tensor.matmul(p_ps[:, c * 512:(c + 1) * 512],
                                 w2[:, c * 128:(c + 1) * 128], un[:],
                                 start=True, stop=True)
            # out_sb cols (g, c, n); the Act drain does the (c,g)->(g,c)
            # reorder so the DMA balances to 3 dims
            out_sb = sb.tile([128, 1024], BF16, tag="out", bufs=2, name=f"out{i}")
            nc.scalar.activation(
                out_sb[:].rearrange("p (g c n) -> p g c n", g=G, c=2),
                p_ps[:].rearrange("p (c g n) -> p g c n", c=2, g=G),
                AF.Identity)
            nc.sync.dma_start(
                outT[b0:b0 + G].rearrange("g (c p) n -> p g c n", p=128),
                out_sb[:].rearrange("p (g c n) -> p g c n", g=G, c=2))

        # software pipeline; emission order per iter =
        #   load(i+3) | proj(i+2) | c2(i-1) | c3(i-2) | b(i+1) | c1(i)
        # so every PE instruction's inputs were produced >= 1 iteration
        # earlier and the in-order PE queue never waits on DMA or slow
        # cross-engine chains.
        N = NGRP_RUN
        for j in range(min(3, N)):
            stage_load(j)
        stage_proj(0)
        if N > 1:
            stage_proj(1)
        stage_b(0)
        for i in range(N):
            if i + 3 < N:
                stage_load(i + 3)
            if i + 2 < N:
                stage_proj(i + 2)
            if i >= 1:
                stage_c2(i - 1)
            if i >= 2:
                stage_c3(i - 2)
            if i + 1 < N:
                stage_b(i + 1)
            stage_c1(i)
        stage_c2(N - 1)
        if N >= 2:
            stage_c3(N - 2)
        stage_c3(N - 1)

    nc.compile()
    return nc


_NC = None


def _get_nc():
    global _NC
    if _NC is None:
        _NC = build_nc()
    return _NC


def _host_consts(W1, b1, Wqkv, bqkv, W2, b2, bias_table):
    import ml_dtypes
    BFnp = ml_dtypes.bfloat16
    Wq, Wk, Wv = Wqkv[:, :SF], Wqkv[:, SF:2 * SF], Wqkv[:, 2 * SF:]
    bq, bv = bqkv[:SF], bqkv[2 * SF:]
    # k bias (bqkv[SF:2SF]) shifts every score of a query by the same amount
    # per (q, h) -> softmax invariant -> dropped.
    wbq = (W1 @ Wq) * SCALE
    bbq = (b1 @ Wq + bq) * SCALE
    c2 = (bv @ W2 + b2).astype(np.float32)
    bias = bias_table[_rel_pos_index()]            # (NB=q, NS=k, H)
    # expb[64*h2+k, t*512+g*256+q] = exp(bias[q, k, 2t+h2]), same for both g
    biasT = np.zeros((128, 1024), np.float32)
    for t in range(2):
        for h2 in range(2):
            bT = bias[:, :, 2 * t + h2].T          # (k, q)
            for g in range(G):
                biasT[64 * h2:64 * h2 + 64,
                      t * 512 + g * 256:t * 512 + (g + 1) * 256] = bT
    # ones4[64*h2:64*(h2+1), t*128 + 32*h : t*128 + 32*h + 32] = 1 for
    # h = 2t + h2: the z matmul for pair t then writes z[head f//32] to
    # every feature partition f of that pair's quarter.
    ones4 = np.zeros((128, 256), np.float32)
    for t in range(2):
        for h2 in range(2):
            h = 2 * t + h2
            ones4[64 * h2:64 * h2 + 64, t * 128 + 32 * h:t * 128 + 32 * h + 32] = 1.0
    bvec = np.zeros((128, 1), np.float32)
    bvec[:, 0] = bbq
    consts = dict(wbq=np.ascontiguousarray(wbq.astype(BFnp)),
                  wk=np.ascontiguousarray(Wk.astype(BFnp)),
                  wv=np.ascontiguousarray(Wv.astype(BFnp)),
                  w2=np.ascontiguousarray(W2.astype(BFnp)),
                  expb=np.exp(biasT).astype(BFnp),
                  ones4=ones4.astype(BFnp),
                  bvec=bvec, zeros=np.zeros((128, 512), BFnp))
    return consts, c2


def make_in_maps(big_x, small_x, W1, b1, Wqkv, bqkv, W2, b2, bias_table):
    import ml_dtypes
    BFnp = ml_dtypes.bfloat16
    consts, c2 = _host_consts(
        np.asarray(W1, np.float32), np.asarray(b1, np.float32),
        np.asarray(Wqkv, np.float32), np.asarray(bqkv, np.float32),
        np.asarray(W2, np.float32), np.asarray(b2, np.float32),
        np.asarray(bias_table, np.float32))
    big_x = np.asarray(big_x, np.float32)
    small_x = np.asarray(small_x, np.float32)
    in_maps = []
    for c in range(NCORES):
        sl = slice(c * BLOC, (c + 1) * BLOC)
        m = dict(consts)
        m["bigxT"] = np.ascontiguousarray(big_x[sl].transpose(0, 2, 1).astype(BFnp))
        m["smallxT"] = np.ascontiguousarray(small_x[sl].transpose(0, 2, 1).astype(BFnp))
        in_maps.append(m)
    return in_maps, c2


def gather_out(results, c2):
    # outT is (BLOC, BF, NB) bf16; epilogue bias c2 (per BF feature) added here
    outs = [(r["outT"].astype(np.float32) + c2[None, :, None]).transpose(0, 2, 1)
            for r in results]
    return np.ascontiguousarray(np.concatenate(outs, axis=0), dtype=np.float32)


def run(inputs, **kw):
    nc = _get_nc()
    in_maps, c2 = make_in_maps(**inputs)
    res = run_bass_kernel_spmd(nc, in_maps, core_ids=list(range(NCORES)), **kw)
    res.c2 = c2
    return res


def kernel(**inputs):
    res = run(inputs)
    return gather_out(res.results, res.c2)
